# revision 29
# baseline (speedup 1.0000x reference)
"""Self-contained Trainium2 Bass kernel for a 6-layer dense transformer.

Model (from reference): DIM=1024, DEPTH=6, HEADS=16, FF=4096, x [2,1024,1024],
relative_position_bias [1,16,1024,1024], pre-norm attention+FFN, exact GELU.

Strategy: 8-way sequence sharding for all weight matmuls + 8-way HEAD sharding
for attention, connected by AllToAll collectives, software-pipelined across the
two batches so every collective (and the attention compute it feeds) hides
inside the other batch's DMA-bound FFN phase.

Precision plan (validated host-side against the 2e-2 rel-err budget):
  - Attention path in fp8(e4m3) with DoubleRow matmuls: w_qkv/w_out stored
    x64 in fp8; h1 (LN1 out) fp8; q,k transported fp8 in a [32ch,2,...]
    layout so scores run DoubleRow; v,o transported fp8 (o carries the x64
    v-scale); wout output descaled by 2^-12 in the residual STT; q*k's
    2^12 folds into the softmax exp scale.
  - FFN stays bf16 end-to-end (fp8 there measured 2.7e-2 -- over budget).
    w1/w2 get partial SBUF residency (first R chunks load once per layer).

Per core: owns rows [c*128,(c+1)*128) of BOTH batches (xT [1024, 256] f32,
channel-major) and heads {2c, 2c+1} of both batches for attention.

Scheduling: per layer two "s3" phases (batch 0 then batch 1), each DMA-bound
on its w1/w2 stream. attn(l, 1) is emitted chunk-interleaved inside s3(l, 0);
attn(l+1, 0) inside s3(l, 1). Weight/param DMAs live alone on the SP queue so
collective-gated gather DMAs (scalar queue) never head-block them. wqkv/wout
are loaded once per layer (resident rings) at the tail of the preceding phase.
"""
import sys
sys.path.insert(0, "/opt/trn_rl_repo")

import numpy as np

import concourse.bass as bass
import concourse.tile as tile
from concourse import bacc, mybir

P = 128
D = 1024
DT = 8            # D / P
DEPTH = 6
HEADS = 16
HL = 2            # heads per core
DH = 64
FF = 4096
FFT = 32          # FF / P
B = 2
RB = 128          # rows per core per batch
R2 = 256          # rows per core total
SEQ = 1024
N_CORES = 8
EPS = 1e-5
SCALE = DH ** -0.5
RG8 = [[0, 1, 2, 3, 4, 5, 6, 7]]

WS = 32.0         # fp8 weight scale for w_qkv / w_out (e4m3 max 240:
                  # keeps q/k ~20 std and o*WS peaks ~110, well clear)
DSC = 1.0 / (WS * WS)   # descale after two chained xWS fp8 factors

F32 = mybir.dt.float32
BF16 = mybir.dt.bfloat16
F8 = mybir.dt.float8e4
AX = mybir.AluOpType
AF = mybir.ActivationFunctionType
DR = mybir.MatmulPerfMode.DoubleRow

NQKV_CH = 12
NOUT_CH = 4
NW1_CH = 16
R1 = 8            # w1 chunks resident across the two batch phases
R2W = 4           # w2 chunks resident across the two batch phases
AV_LAG = 3        # scores run this many key-blocks ahead of AV
H0_AT = "cp0"    # where the fused attention block is emitted
H1_AT = "cp1"    # where the Ao staging+fire is emitted


def _bcast_mid(ap, n):
    """View a [P, N] AP as [P, n, N] with a 0-stride middle dim."""
    return bass.AP(tensor=ap.tensor, offset=ap.offset,
                   ap=[list(ap.ap[0]), [0, n], list(ap.ap[1])])


def build_nc(has_bout=False):
    nc = bacc.Bacc("TRN2", target_bir_lowering=False, debug=False,
                   num_devices=N_CORES)

    xT_ext = nc.dram_tensor("xT", [D, R2], F32, kind="ExternalInput")
    ebT_ext = nc.dram_tensor("ebT", [HL, DT, P, SEQ], BF16,
                             kind="ExternalInput")
    w_qkv_ext = nc.dram_tensor("w_qkv", [DEPTH, NQKV_CH, P, DT, 2 * P], F8,
                               kind="ExternalInput")
    w_out_ext = nc.dram_tensor("w_out", [DEPTH, NOUT_CH, P, DT, 2 * P], F8,
                               kind="ExternalInput")
    w1_ext = nc.dram_tensor("w1", [DEPTH, NW1_CH, P, DT, 2 * P], BF16,
                            kind="ExternalInput")
    w2_ext = nc.dram_tensor("w2", [DEPTH, 4, 4, P, DT, 2 * P], BF16,
                            kind="ExternalInput")
    b_out_ext = nc.dram_tensor("b_out", [DEPTH, D], F32, kind="ExternalInput")
    ln1_g_ext = nc.dram_tensor("ln1_g", [DEPTH, D], F32, kind="ExternalInput")
    ln1_b_ext = nc.dram_tensor("ln1_b", [DEPTH, D], F32, kind="ExternalInput")
    ln2_g_ext = nc.dram_tensor("ln2_g", [DEPTH, D], F32, kind="ExternalInput")
    ln2_b_ext = nc.dram_tensor("ln2_b", [DEPTH, D], F32, kind="ExternalInput")
    b1_ext = nc.dram_tensor("b1", [DEPTH, FF], F32, kind="ExternalInput")
    b2_ext = nc.dram_tensor("b2", [DEPTH, D], F32, kind="ExternalInput")
    outT_ext = nc.dram_tensor("outT", [D, R2], F32, kind="ExternalOutput")

    import os
    DEBUG_DUMP = bool(os.environ.get("KERNEL_DEBUG_DUMP"))
    dbg_exts = {}

    def dbg(name, ap):
        if not DEBUG_DUMP or name in dbg_exts:
            return
        shape = [d[1] for d in ap.ap]
        ext = nc.dram_tensor(f"dbg_{name}", shape, ap.dtype,
                             kind="ExternalOutput")
        dbg_exts[name] = ext
        nc.sync.dma_start(out=ext.ap(), in_=ap)

    from contextlib import ExitStack
    with tile.TileContext(nc) as tc, ExitStack() as ctx:
        ep = ctx.enter_context
        singles = ep(tc.tile_pool(name="singles", bufs=1))
        params = ep(tc.tile_pool(name="params", bufs=3))
        statp = ep(tc.tile_pool(name="stat", bufs=1))
        vecp = ep(tc.tile_pool(name="vecp", bufs=2))
        qkTp = ep(tc.tile_pool(name="qkTp", bufs=1))
        vpp = ep(tc.tile_pool(name="vpp", bufs=1))
        attnp = ep(tc.tile_pool(name="attnp", bufs=9))
        oTmp = ep(tc.tile_pool(name="oTmp", bufs=2))
        oFp = ep(tc.tile_pool(name="oFp", bufs=2))
        hTp = ep(tc.tile_pool(name="hTp", bufs=3))
        gTp = ep(tc.tile_pool(name="gTp", bufs=1))
        qkvSp = ep(tc.tile_pool(name="qkvSp", bufs=1))
        wqp = ep(tc.tile_pool(name="wqp", bufs=NQKV_CH))
        woutp = ep(tc.tile_pool(name="woutp", bufs=NOUT_CH))
        wcp = ep(tc.tile_pool(name="wcp", bufs=7))
        w1res = ep(tc.tile_pool(name="w1res", bufs=R1))
        w2res = ep(tc.tile_pool(name="w2res", bufs=R2W))
        psS = ep(tc.tile_pool(name="psS", bufs=2, space="PSUM"))
        psO = ep(tc.tile_pool(name="psO", bufs=2, space="PSUM"))
        psM = ep(tc.tile_pool(name="psM", bufs=3, space="PSUM"))
        psT = ep(tc.tile_pool(name="psT", bufs=1, space="PSUM"))
        dram = ep(tc.tile_pool(name="dram", bufs=8, space="DRAM"))

        # ---- persistent tiles ----
        xT = singles.tile([P, DT, R2], F32, tag="xT")
        EB = singles.tile([P, HL, DT, SEQ], BF16, tag="EB")
        ones_red = singles.tile([P, 1], BF16, tag="ones_red")
        ones_k1 = singles.tile([1, P], BF16, tag="ones_k1")
        ones_f = singles.tile([1, P], F32, tag="ones_f")
        nc.vector.memset(ones_red[:], 1.0)
        nc.vector.memset(ones_k1[:], 1.0)
        nc.vector.memset(ones_f[:], 1.0)

        nc.sync.dma_start(
            out=xT[:], in_=xT_ext.ap().rearrange("(t p) r -> p t r", p=P))

        # ---- caches of loaded tiles ----
        param_cache = {}
        wq_tiles = {}
        wo_tiles = {}
        aq_bufs = {}
        ao_bufs = {}
        w1res_tiles = {}
        w2res_tiles = {}

        def load_wq(l):
            for ch in range(NQKV_CH):
                wc = wqp.tile([P, DT, 2 * P], F8, tag="wq",
                              name=f"wq_{l}_{ch}")
                nc.sync.dma_start(out=wc[:], in_=w_qkv_ext.ap()[l, ch])
                wq_tiles[(l, ch)] = wc

        def load_wout(l):
            for ch in range(NOUT_CH):
                wc = woutp.tile([P, DT, 2 * P], F8, tag="wout",
                                name=f"wo_{l}_{ch}")
                nc.sync.dma_start(out=wc[:], in_=w_out_ext.ap()[l, ch])
                wo_tiles[(l, ch)] = wc

        def get_params(l):
            if l in param_cache:
                return param_cache[l]
            pr = {}
            pr["g1"] = params.tile([P, DT], F32, tag="g1", name=f"g1_{l}")
            pr["b1p"] = params.tile([P, DT], F32, tag="b1p", name=f"b1p_{l}")
            pr["g2"] = params.tile([P, DT], F32, tag="g2", name=f"g2_{l}")
            pr["b2p"] = params.tile([P, DT], F32, tag="b2p", name=f"b2p_{l}")
            pr["bo"] = params.tile([P, DT], F32, tag="bo", name=f"bo_{l}")
            pr["bf"] = params.tile([P, FFT], F32, tag="bf", name=f"bf_{l}")
            pr["b2f"] = params.tile([P, DT], F32, tag="b2f", name=f"b2f_{l}")
            nc.sync.dma_start(out=pr["g1"][:], in_=ln1_g_ext.ap()[l].rearrange("(t p) -> p t", p=P))
            nc.sync.dma_start(out=pr["b1p"][:], in_=ln1_b_ext.ap()[l].rearrange("(t p) -> p t", p=P))
            nc.sync.dma_start(out=pr["g2"][:], in_=ln2_g_ext.ap()[l].rearrange("(t p) -> p t", p=P))
            nc.sync.dma_start(out=pr["b2p"][:], in_=ln2_b_ext.ap()[l].rearrange("(t p) -> p t", p=P))
            nc.sync.dma_start(out=pr["bo"][:], in_=b_out_ext.ap()[l].rearrange("(t p) -> p t", p=P))
            nc.sync.dma_start(out=pr["bf"][:], in_=b1_ext.ap()[l].rearrange("(t p) -> p t", p=P))
            nc.sync.dma_start(out=pr["b2f"][:], in_=b2_ext.ap()[l].rearrange("(t p) -> p t", p=P))
            param_cache[l] = pr
            return pr

        def ln_alloc(tag):
            xb = statp.tile([P, DT, RB], BF16, tag="xb", name=f"xb_{tag}")
            sq = statp.tile([P, DT, RB], BF16, tag="sq", name=f"sq_{tag}")
            st = psT.tile([33, RB], F32, tag="st", name=f"st_{tag}")
            return xb, sq, st

        def ln_contrib(stt, t, b):
            xb, sq, st = stt
            nc.vector.tensor_copy(xb[:, t], xT[:, t, b * RB:(b + 1) * RB])
            nc.vector.tensor_mul(sq[:, t], xb[:, t], xb[:, t])
            nc.tensor.matmul(st[0:1], ones_red[:], xb[:, t],
                             start=(t == 0), stop=(t == DT - 1))
            nc.tensor.matmul(st[32:33], ones_red[:], sq[:, t],
                             start=(t == 0), stop=(t == DT - 1))

        def ln_finish(stt, g_sb, b_sb, out_h, tag):
            xb, sq, st = stt
            mu = vecp.tile([1, RB], F32, tag="mu", name=f"mu_{tag}")
            var = vecp.tile([1, RB], F32, tag="var", name=f"var_{tag}")
            ms = vecp.tile([1, RB], F32, tag="ms", name=f"ms_{tag}")
            rstd = vecp.tile([1, RB], F32, tag="rstd", name=f"rstd_{tag}")
            nc.vector.tensor_scalar_mul(mu[:], st[0:1], 1.0 / D)
            nc.vector.tensor_scalar_mul(var[:], st[32:33], 1.0 / D)
            nc.vector.tensor_mul(ms[:], mu[:], mu[:])
            nc.vector.tensor_sub(var[:], var[:], ms[:])
            nc.vector.tensor_scalar_add(var[:], var[:], EPS)
            nc.scalar.activation(var[:], var[:], AF.Sqrt)
            nc.vector.reciprocal(rstd[:], var[:])
            ps_mu = psS.tile([P, RB], F32, tag="s", name=f"psmu_{tag}")
            ps_rs = psS.tile([P, RB], F32, tag="s", name=f"psrs_{tag}")
            nc.tensor.matmul(ps_mu[:], ones_f[:], mu[:], start=True, stop=True)
            nc.tensor.matmul(ps_rs[:], ones_f[:], rstd[:], start=True, stop=True)
            mub = statp.tile([P, RB], BF16, tag="mub", name=f"mub_{tag}")
            rsb = statp.tile([P, RB], BF16, tag="rsb", name=f"rsb_{tag}")
            nc.vector.tensor_copy(mub[:], ps_mu[:])
            nc.vector.tensor_copy(rsb[:], ps_rs[:])
            nc.vector.tensor_sub(xb[:], xb[:], _bcast_mid(mub[:], DT))
            nc.vector.tensor_mul(xb[:], xb[:], _bcast_mid(rsb[:], DT))
            for t in range(DT):
                nc.vector.tensor_scalar(
                    out_h[:, t], xb[:, t], g_sb[:, t:t + 1], b_sb[:, t:t + 1],
                    op0=AX.mult, op1=AX.add)

        def get_aq(l, b):
            if (l, b) not in aq_bufs:
                ai = dram.tile([N_CORES, P, 3, P], F8, tag="aq_in",
                               name=f"aqi_{l}_{b}")
                ao = dram.tile([N_CORES, P, 3, P], F8, tag="aq_out",
                               name=f"aqo_{l}_{b}")
                aq_bufs[(l, b)] = (ai, ao)
            return aq_bufs[(l, b)]

        def get_ao(l, b):
            if (l, b) not in ao_bufs:
                ai = dram.tile([N_CORES, P, P], F8, tag="ao_in",
                               name=f"aoi_{l}_{b}")
                ao = dram.tile([N_CORES, P, P], F8, tag="ao_out",
                               name=f"aoo_{l}_{b}")
                ao_bufs[(l, b)] = (ai, ao)
            return ao_bufs[(l, b)]

        def s1_qkv(l, b, h1):
            """QKV (fp8 DoubleRow) for layer l, batch b; stage + fire Aq.

            h1 is fp8 [P, DT, RB]. Weight ring holds all of wqkv[l] (loaded
            by the b==0 call). Column blocks destination-major so each
            destination's q/k/v triple finishes together and its staging DMA
            fires early.
            """
            qkvS = qkvSp.tile([P, N_CORES, 3, P], F8, tag="qkvS",
                              name=f"qkvS_{l}_{b}")
            aq_i, aq_o = get_aq(l, b)
            if b == 0:
                load_wq(l)
            for d in range(N_CORES):
                for kind in range(3):
                    c = kind * 8 + d
                    ch, sub = c // 2, c % 2
                    wc = wq_tiles[(l, ch)]
                    ps = psM.tile([P, P], F32, tag="mm",
                                  name=f"psq_{l}_{b}_{c}")
                    if kind < 2:  # q, k: channel-major [ch, row]
                        for kp in range(DT // 2):
                            nc.tensor.matmul(
                                ps[:],
                                wc[:, 2 * kp:2 * kp + 2, sub * P:(sub + 1) * P],
                                h1[:, 2 * kp:2 * kp + 2, :],
                                start=(kp == 0), stop=(kp == DT // 2 - 1),
                                perf_mode=DR)
                    else:         # v: row-major [row, ch]
                        for kp in range(DT // 2):
                            nc.tensor.matmul(
                                ps[:], h1[:, 2 * kp:2 * kp + 2, :],
                                wc[:, 2 * kp:2 * kp + 2, sub * P:(sub + 1) * P],
                                start=(kp == 0), stop=(kp == DT // 2 - 1),
                                perf_mode=DR)
                    if c % 2 == 0:
                        nc.vector.tensor_copy(qkvS[:, d, kind, :], ps[:])
                    else:
                        nc.scalar.activation(qkvS[:, d, kind, :], ps[:],
                                             AF.Copy)
                eng = nc.gpsimd if d % 2 == 0 else nc.scalar
                eng.dma_start(
                    out=aq_i[d].rearrange("p k r -> p (k r)"),
                    in_=qkvS[:, d].rearrange("p k r -> p (k r)"))
            if l == 0 and b == 0:
                dbg("qkvS00", qkvS[:])
            nc.gpsimd.collective_compute(
                "AllToAll", AX.bypass, replica_groups=RG8,
                ins=[aq_i[:]], outs=[aq_o[:]])

        def attn_parts(l, b):
            """Attention for my 2 heads of batch b, split into emission chunks.

            Returns [gather, head0, head1_and_fire] closures.
            """
            st = {}

            def gather():
                _, aq_o = get_aq(l, b)
                # qkT [p=hl*32+pl, kind(q/k), kt, ktk, row] fp8: partition p
                # holds channel kt*32+pl of head hl -- DoubleRow layout.
                qkT = qkTp.tile([64, 2, 2, DT, P], F8, tag="qkT",
                                name=f"qkT_{l}_{b}")
                Vp = vpp.tile([P, DT, HL, DH + 1], F8, tag="Vp",
                              name=f"Vp_{l}_{b}")
                nc.vector.memset(Vp[:, :, :, DH:DH + 1], 1.0)
                for hl in range(HL):
                    for kind in range(2):
                        for kt in range(2):
                            co = hl * 64 + kt * 32
                            eng = (nc.scalar if (kind + kt) % 2 == 0
                                   else nc.gpsimd)
                            eng.dma_start(
                                out=qkT[hl * 32:(hl + 1) * 32, kind, kt],
                                in_=aq_o[:, co:co + 32, kind, :]
                                .rearrange("c pl r -> pl c r"))
                for hh in range(HL):
                    nc.scalar.dma_start(
                        out=Vp[:, :, hh, 0:DH],
                        in_=aq_o[:, :, 2, hh * DH:(hh + 1) * DH]
                        .rearrange("c p j -> p c j"))
                if l == 0 and b == 0:
                    dbg("qkT00", qkT[:])
                    dbg("Vp00", Vp[:])
                st["qkT"], st["Vp"] = qkT, Vp
                st["oTm"] = oTmp.tile([P, DT, P], F8, tag="oTm",
                                      name=f"oTm_{l}_{b}")

            def heads():
                """Both heads fused, rh-major: one pass over key-blocks per
                query-half keeps the Act exp stream dense and takes head1 off
                the FFN2-interleaved PE stream."""
                qkT, Vp = st["qkT"], st["Vp"]
                for rh in range(2):
                    ps_os = [psO.tile([DH + 1, 512], F32, tag="o",
                                      name=f"psO_{l}_{b}_{hl}_{rh}")
                             for hl in range(HL)]
                    ats = {}
                    for ktk in range(DT + AV_LAG):
                        if ktk < DT:
                            for hl in range(HL):
                                hs = slice(hl * 32, (hl + 1) * 32)
                                at = attnp.tile([P, 512], BF16, tag="at",
                                                name=f"at_{l}_{b}_{hl}_{ktk}_{rh}")
                                ps_s = psS.tile([P, 512], F32, tag="s",
                                                name=f"psS_{l}_{b}_{hl}_{ktk}_{rh}")
                                nc.tensor.matmul(
                                    ps_s[:],
                                    qkT[hs, 1, :, ktk, :],
                                    qkT[hs, 0, :, rh * 4:(rh + 1) * 4, :],
                                    start=True, stop=True, perf_mode=DR)
                                nc.scalar.activation(at[:], ps_s[:], AF.Exp,
                                                     scale=SCALE * DSC)
                                nc.vector.tensor_mul(
                                    at[:], at[:],
                                    EB[:, hl, ktk, rh * 512:(rh + 1) * 512])
                                ats[(ktk, hl)] = at
                        pk = ktk - AV_LAG
                        if pk >= 0:
                            for hl in range(HL):
                                pat = ats.pop((pk, hl))
                                nc.tensor.matmul(ps_os[hl][:],
                                                 Vp[:, pk, hl, :], pat[:],
                                                 start=(pk == 0),
                                                 stop=(pk == DT - 1))
                    for hl in range(HL):
                        rec = vecp.tile([1, 512], F32, tag="rec",
                                        name=f"rec_{l}_{b}_{hl}_{rh}")
                        rec16 = vecp.tile([1, 512], BF16, tag="rec16",
                                          name=f"rec16_{l}_{b}_{hl}_{rh}")
                        nc.vector.reciprocal(rec[:], ps_os[hl][DH:DH + 1])
                        nc.vector.tensor_copy(rec16[:], rec[:])
                        ps_rb = psS.tile([DH, 512], F32, tag="s",
                                         name=f"psrb_{l}_{b}_{hl}_{rh}")
                        nc.tensor.matmul(ps_rb[:], ones_k1[0:1, 0:DH],
                                         rec16[:], start=True, stop=True)
                        hof = hl * DH
                        dst = st["oTm"][hof:hof + DH, rh * 4:(rh + 1) * 4, :] \
                            .rearrange("p a b -> p (a b)")
                        # normalize via bf16 staging: fp8 can't hold the raw
                        # numerator (overflows 240), and DVE can't read two
                        # PSUM operands in one op
                        ntmp = vecp.tile([DH, 512], BF16, tag="ntmp",
                                         name=f"ntmp_{l}_{b}_{hl}_{rh}")
                        nc.vector.tensor_copy(ntmp[:], ps_os[hl][0:DH])
                        nc.vector.tensor_mul(dst, ntmp[:], ps_rb[:])

            def stage_head(hl):
                ao_i, _ = get_ao(l, b)
                hof = hl * DH
                nc.scalar.dma_start(
                    out=ao_i[:, hof:hof + DH, :].rearrange("d p r -> p d r"),
                    in_=st["oTm"][hof:hof + DH])

            def fire():
                ao_i, ao_o = get_ao(l, b)
                nc.gpsimd.collective_compute(
                    "AllToAll", AX.bypass, replica_groups=RG8,
                    ins=[ao_i[:]], outs=[ao_o[:]])

            def h0():
                heads()
                stage_head(0)

            def h1_fire():
                stage_head(1)
                fire()

            return gather, h0, h1_fire

        def s3(l, b, host=None, pre=None, defer_tail=False):
            """w_out + residual + LN2 + FFN (+ LN1/QKV for l+1) for batch b.

            host: attn_parts() tuple emitted interleaved into this phase.
            pre: deferred LN1-finish+QKV closure from the previous phase,
            emitted at this phase's head so its LN serial chain overlaps the
            Ao-collective wait instead of exposing at the prior phase's tail.
            """
            if pre:
                pre()
            pr = get_params(l)
            bc = slice(b * RB, (b + 1) * RB)
            _, ao_o = get_ao(l, b)
            oF = oFp.tile([P, DT, P], F8, tag="oF", name=f"oF_{l}_{b}")
            nc.scalar.dma_start(
                out=oF[:], in_=ao_o[:].rearrange("c p r -> p c r"))

            if l == 0 and b == 0:
                dbg("oF00", oF[:])
            ln2_st = ln_alloc(f"l{l}b{b}o")
            for ch in range(NOUT_CH):
                wc = wo_tiles[(l, ch)]
                for sub in range(2):
                    oc = ch * 2 + sub
                    ps = psM.tile([P, P], F32, tag="mm",
                                  name=f"pso_{l}_{b}_{oc}")
                    for kp in range(DT // 2):
                        nc.tensor.matmul(
                            ps[:],
                            wc[:, 2 * kp:2 * kp + 2, sub * P:(sub + 1) * P],
                            oF[:, 2 * kp:2 * kp + 2, :],
                            start=(kp == 0), stop=(kp == DT // 2 - 1),
                            perf_mode=DR)
                    nc.vector.scalar_tensor_tensor(
                        out=xT[:, oc, bc], in0=ps[:], scalar=DSC,
                        in1=xT[:, oc, bc], op0=AX.mult, op1=AX.add)
                    if has_bout:
                        nc.vector.tensor_scalar(
                            xT[:, oc, bc], xT[:, oc, bc],
                            pr["bo"][:, oc:oc + 1], None, op0=AX.add)
                    ln_contrib(ln2_st, oc, b)

            if l == 0 and b == 0:
                dbg("xTwo0", xT[:])
            h2 = hTp.tile([P, DT, RB], BF16, tag="h2", name=f"h2_{l}_{b}")
            ln_finish(ln2_st, pr["g2"], pr["b2p"], h2, f"l{l}b{b}2")
            if l == 0 and b == 0:
                dbg("h200", h2[:])

            gT = gTp.tile([P, FFT, RB], BF16, tag="gT", name=f"gT_{l}_{b}")
            for ch in range(NW1_CH):
                if ch < R1:
                    if b == 0:
                        wc = w1res.tile([P, DT, 2 * P], BF16, tag="w1r",
                                        name=f"w1r_{l}_{ch}")
                        nc.sync.dma_start(out=wc[:], in_=w1_ext.ap()[l, ch])
                        w1res_tiles[ch] = wc
                    else:
                        wc = w1res_tiles[ch]
                else:
                    wc = wcp.tile([P, DT, 2 * P], BF16, tag="wc",
                                  name=f"wc1_{l}_{b}_{ch}")
                    nc.sync.dma_start(out=wc[:], in_=w1_ext.ap()[l, ch])
                for sub in range(2):
                    f = ch * 2 + sub
                    ps = psM.tile([P, P], F32, tag="mm",
                                  name=f"psf_{l}_{b}_{f}")
                    for kt in range(DT):
                        nc.tensor.matmul(ps[:], wc[:, kt, sub * P:(sub + 1) * P],
                                         h2[:, kt], start=(kt == 0),
                                         stop=(kt == DT - 1))
                    nc.scalar.activation(gT[:, f], ps[:], AF.Gelu,
                                         bias=pr["bf"][:, f:f + 1])

            if b == 1 and l < DEPTH - 1:
                # next layer's resident w_out: load mid-phase so its DMA
                # doesn't collide with the boundary staging/gather burst
                load_wout(l + 1)
            if host:
                host[0]()   # attn gathers (scalar queue; waits its Aq)
                if H0_AT == "ffn1":
                    host[1]()
                if H1_AT == "ffn1":
                    host[2]()

            last = l == DEPTH - 1
            if not last:
                ln1_st = ln_alloc(f"l{l + 1}b{b}a")
            for cp in range(4):
                pss = [psM.tile([P, P], F32, tag="mm",
                                name=f"ps2_{l}_{b}_{cp}_{i}") for i in range(2)]
                for ktg in range(4):
                    w2i = cp * 4 + ktg
                    if w2i < R2W:
                        if b == 0:
                            wc = w2res.tile([P, DT, 2 * P], BF16, tag="w2r",
                                            name=f"w2r_{l}_{w2i}")
                            nc.sync.dma_start(out=wc[:], in_=w2_ext.ap()[l, cp, ktg])
                            w2res_tiles[w2i] = wc
                        else:
                            wc = w2res_tiles[w2i]
                    else:
                        wc = wcp.tile([P, DT, 2 * P], BF16, tag="wc",
                                      name=f"wc2_{l}_{b}_{cp}_{ktg}")
                        nc.sync.dma_start(out=wc[:], in_=w2_ext.ap()[l, cp, ktg])
                    for sub in range(2):
                        for k8 in range(DT):
                            nc.tensor.matmul(
                                pss[sub][:], wc[:, k8, sub * P:(sub + 1) * P],
                                gT[:, ktg * 8 + k8],
                                start=(ktg == 0 and k8 == 0),
                                stop=(ktg == 3 and k8 == DT - 1))
                for sub in range(2):
                    oc = cp * 2 + sub
                    nc.vector.scalar_tensor_tensor(
                        out=xT[:, oc, bc], in0=pss[sub][:],
                        scalar=pr["b2f"][:, oc:oc + 1],
                        in1=xT[:, oc, bc], op0=AX.add, op1=AX.add)
                    if not last:
                        ln_contrib(ln1_st, oc, b)
                if host and H0_AT == f"cp{cp}":
                    host[1]()
                if host and H1_AT == f"cp{cp}":
                    host[2]()

            if not last:
                prn = get_params(l + 1)

                def tail():
                    h1 = hTp.tile([P, DT, RB], F8, tag="h1",
                                  name=f"h1_{l + 1}_{b}")
                    ln_finish(ln1_st, prn["g1"], prn["b1p"], h1,
                              f"l{l + 1}b{b}1")
                    s1_qkv(l + 1, b, h1)
                if defer_tail:
                    return tail
                tail()
            else:
                # final layer: this batch's half of x is complete -- write it
                # out now so the b0 half overlaps the b1 phase
                nc.sync.dma_start(
                    out=outT_ext.ap()[:, b * RB:(b + 1) * RB]
                    .rearrange("(t p) r -> p t r", p=P),
                    in_=xT[:, :, bc])

        # ---- prologue: LN1(0) + QKV(0); batch 1's QKV after the first
        # attention so Ao(0,0) isn't queued behind Aq(0,1) on the CC device --
        def s1_boot(b):
            pr = get_params(0)
            st0 = ln_alloc(f"l0b{b}a")
            for t in range(DT):
                ln_contrib(st0, t, b)
            h1 = hTp.tile([P, DT, RB], F8, tag="h1", name=f"h1_0_{b}")
            ln_finish(st0, pr["g1"], pr["b1p"], h1, f"l0b{b}1")
            if b == 0:
                dbg("h100", h1[:])
            s1_qkv(0, b, h1)

        s1_boot(0)
        load_wout(0)
        nc.sync.dma_start(
            out=EB[:], in_=ebT_ext.ap().rearrange("h t p s -> p h t s"))

        # ---- first attention (exposed; nothing to hide under) ----
        a00 = attn_parts(0, 0)
        a00[0]()
        a00[1]()
        a00[2]()
        s1_boot(1)

        # ---- main loop: s3(l,0) hosts attn(l,1); s3(l,1) hosts attn(l+1,0).
        # batch-1's LN1/QKV tail is deferred into the next phase's head ----
        pend = None
        for l in range(DEPTH):
            pend = s3(l, 0, host=attn_parts(l, 1), pre=pend)
            host_next = attn_parts(l + 1, 0) if l < DEPTH - 1 else None
            pend = s3(l, 1, host=host_next, pre=pend, defer_tail=True)

    nc.compile()
    return nc


def make_in_maps(inputs):
    import ml_dtypes
    bf16 = ml_dtypes.bfloat16
    f8 = ml_dtypes.float8_e4m3
    x = np.ascontiguousarray(np.asarray(inputs["x"], dtype=np.float32))
    bias = np.asarray(inputs["relative_position_bias"], dtype=np.float32)

    def pack(w, nch, dt, scale=1.0):
        w = np.asarray(w, dtype=np.float32) * scale
        return np.ascontiguousarray(
            w.reshape(DEPTH, DT, P, nch, 2 * P).transpose(0, 3, 2, 1, 4)
            .astype(dt))

    w2 = np.asarray(inputs["w2"], dtype=np.float32)
    w2p = np.ascontiguousarray(
        w2.reshape(DEPTH, 4, DT, P, 4, 2 * P).transpose(0, 4, 1, 3, 2, 5)
        .astype(bf16))

    shared = {
        "w_qkv": pack(inputs["w_qkv"], NQKV_CH, f8, WS),
        "w_out": pack(inputs["w_out"], NOUT_CH, f8, WS),
        "w1": pack(inputs["w1"], NW1_CH, bf16),
        "w2": w2p,
        "b_out": np.ascontiguousarray(inputs["b_out"], dtype=np.float32),
        "ln1_g": np.ascontiguousarray(inputs["ln1_g"], dtype=np.float32),
        "ln1_b": np.ascontiguousarray(inputs["ln1_b"], dtype=np.float32),
        "ln2_g": np.ascontiguousarray(inputs["ln2_g"], dtype=np.float32),
        "ln2_b": np.ascontiguousarray(inputs["ln2_b"], dtype=np.float32),
        "b1": np.ascontiguousarray(inputs["b1"], dtype=np.float32),
        "b2": np.ascontiguousarray(inputs["b2"], dtype=np.float32),
    }
    in_maps = []
    for c in range(N_CORES):
        m = dict(shared)
        rs = slice(c * RB, (c + 1) * RB)
        m["xT"] = np.ascontiguousarray(
            np.concatenate([x[0, rs, :].T, x[1, rs, :].T], axis=1))
        eb = np.exp(bias[0, 2 * c:2 * c + 2].astype(np.float64)).astype(np.float32)
        # [h, rows, keys] -> [h, keys, rows] -> [h, ktk, key_in_block, row]
        ebT = eb.transpose(0, 2, 1).reshape(HL, DT, P, SEQ)
        m["ebT"] = np.ascontiguousarray(ebT.astype(bf16))
        in_maps.append(m)
    return in_maps


_NC_CACHE = {}


def kernel(**inputs):
    from concourse.bass_utils import run_bass_kernel_spmd
    has_bout = bool(np.any(np.asarray(inputs["b_out"])))
    key = ("nc", has_bout)
    if key not in _NC_CACHE:
        _NC_CACHE[key] = build_nc(has_bout=has_bout)
        _NC_CACHE["nc"] = _NC_CACHE[key]
    nc = _NC_CACHE[key]
    in_maps = make_in_maps(inputs)
    res = run_bass_kernel_spmd(nc, in_maps, core_ids=list(range(N_CORES)))
    out = np.empty((B, SEQ, D), dtype=np.float32)
    for c in range(N_CORES):
        rs = slice(c * RB, (c + 1) * RB)
        r = np.asarray(res.results[c]["outT"])
        out[0, rs, :] = r[:, 0:RB].T
        out[1, rs, :] = r[:, RB:R2].T
    return out
